# revision 25
# baseline (speedup 1.0000x reference)
"""Chamfer distance kernel for Trainium2 (8 NeuronCores, data-parallel over batch).

Per core (NB=2 batches of the global B=16):
  The [N, N] squared-distance matrix is computed tilewise on the TensorEngine
  with a split-precision K=10 fp16 matmul (fp16 hi/lo decomposition of the
  fp32 inputs; fp16 products are exact and accumulate in fp32 PSUM, so the
  result matches fp32 to ~1e-6 absolute while running 4x faster than fp32
  matmuls):
      d2[i,j] = a2_i + b2_j - 2 a_i.b_j
      rows:  [-2h_ax, -2h_ay, -2h_ax, -2h_ay, -2l_ax, -2l_ay, h_a2, l_a2, 1, 1]
      cols:  [ h_bx,   h_by,   l_bx,   l_by,   h_bx,   h_by,   1,   1, h_b2, l_b2]
  Four 128-row tiles are packed into the four 32-row groups of the PE array
  (tile_position) so their matmuls run concurrently.
  ScalarE copies PSUM fp32 -> SBUF fp16 (the d2 values are >= 0 and small, so
  fp16 error is relative, ~2^-11).
  Row direction (pred->target): in-place fp16 min-tree along the free axis +
  reduce_min. Col direction (target->pred): VectorE pre-mins the 4 row-tiles
  pairwise and accumulates into colacc[128, N]; finally PE-transposed 128x128
  blocks + reduce_min. Partition sums via a ones-vector matmul.
Each core returns [NB, 2] partial sums; the host sums across cores and
divides by N*B. Host does O(N) layout prep only; all O(N^2) work is on device.
"""

import os

import numpy as np

# The axon NTFF-profiling hook module (antenv.axon_hooks) is absent in this
# image; if BASS_TRACE happens to be set in the environment, the trace path
# would crash on import. Never trace from the kernel itself.
os.environ["BASS_NEVER_TRACE"] = "1"

import concourse.bass as bass
import concourse.mybir as mybir
from concourse import bacc
from concourse.tile import TileContext
from concourse.masks import make_identity
from concourse.bass_utils import run_bass_kernel_spmd

F32 = mybir.dt.float32
F16 = mybir.dt.float16
AX = mybir.AxisListType
OP = mybir.AluOpType

N_CORES = 8
KR = 10                   # split-precision contraction depth
FBIG = 60000.0            # fp16-representable "infinity"


def build_chamfer(nb: int, n: int) -> bacc.Bacc:
    """Build the per-core Bass program: nb batches of n points (2-D each)."""
    assert n % 512 == 0
    n_m = n // 128            # 128-row tiles
    n_g = n_m // 4            # groups of 4 row-tiles
    JC = 512                  # matmul moving-operand width
    n_j = n // JC

    nc = bacc.Bacc(
        "TRN2", target_bir_lowering=False, debug=False, enable_asserts=False
    )
    # predQ: per group-slot g (0..3): the KR lhsT rows of row-tile m = 4*G + g,
    # at partitions 32g..32g+KR-1, columns G*128..(G+1)*128.
    predQ_d = nc.dram_tensor("predQ", [nb, 4, KR, n // 4], F16, kind="ExternalInput")
    # targQ: the KR rhs rows replicated at partitions 32g..32g+KR-1.
    targQ_d = nc.dram_tensor("targQ", [nb, 4, KR, n], F16, kind="ExternalInput")
    out_d = nc.dram_tensor("out", [nb, 2], F32, kind="ExternalOutput")

    with TileContext(nc) as tc:
        with (
            tc.tile_pool(name="persist", bufs=1) as pp,
            tc.tile_pool(name="sb", bufs=1) as sb,
            tc.tile_pool(name="sbin", bufs=2) as sbin,
            tc.tile_pool(name="sbx", bufs=3) as sbx,
            tc.tile_pool(name="sbc", bufs=2) as sbc,
            tc.tile_pool(name="ps", bufs=2, space="PSUM") as ps,
        ):
            ident = pp.tile([128, 128], F16)
            make_identity(nc, ident)
            ones = pp.tile([128, 1], F32)
            nc.vector.memset(ones, 1.0)

            for b in range(nb):
                predQ = sbin.tile([128, n // 4], F16, tag="predQ")
                targQ = sbin.tile([128, n], F16, tag="targQ")
                for g in range(4):
                    nc.sync.dma_start(
                        predQ[32 * g : 32 * g + KR, :], predQ_d.ap()[b, g]
                    )
                    nc.sync.dma_start(
                        targQ[32 * g : 32 * g + KR, :], targQ_d.ap()[b, g]
                    )

                colacc = sb.tile([128, n], F16, tag="colacc")
                nc.gpsimd.memset(colacc, FBIG)
                # fin columns: [0, n_m) = rowmins, [n_m, 2*n_m) = colmins
                fin = sb.tile([128, 2 * n_m], F32, tag="fin")

                for G in range(n_g):
                    # xg: fp16 d2 rows for the 4 row-tiles of this group
                    xg = sbx.tile([128, 4, n], F16, tag="xg")
                    for j in range(n_j):
                        pst = ps.tile([128, 4 * JC], F32, tag="mm")
                        for g in range(4):
                            nc.tensor.matmul(
                                pst[:, g * JC : (g + 1) * JC],
                                predQ[32 * g : 32 * g + KR, G * 128 : (G + 1) * 128],
                                targQ[32 * g : 32 * g + KR, j * JC : (j + 1) * JC],
                                start=True,
                                stop=True,
                                tile_position=(32 * g, 0),
                            )
                        # PSUM fp32 [128, 4*JC] -> SBUF fp16, strided over xg.
                        nc.scalar.copy(xg[:, :, j * JC : (j + 1) * JC], pst)

                    # col direction: pre-min the 4 row-tiles pairwise (before
                    # the in-place row tree destroys xg), then fold into
                    # colacc. c2 is computed in-place in c1's first half.
                    c1 = sbc.tile([128, 2, n], F16, tag="c1")
                    nc.vector.tensor_tensor(
                        c1, xg[:, 0:2, :], xg[:, 2:4, :], op=OP.min
                    )
                    nc.vector.tensor_tensor(
                        c1[:, 0, :], c1[:, 0, :], c1[:, 1, :], op=OP.min
                    )
                    nc.vector.tensor_tensor(colacc, colacc, c1[:, 0, :], op=OP.min)

                    # row direction: in-place min-tree over [128, 4, n]
                    w = n // 2
                    nc.vector.tensor_tensor(
                        xg[:, :, 0:w], xg[:, :, 0:w], xg[:, :, w : 2 * w], op=OP.min
                    )
                    while w > 256:
                        w //= 2
                        nc.vector.tensor_tensor(
                            xg[:, :, 0:w], xg[:, :, 0:w], xg[:, :, w : 2 * w],
                            op=OP.min,
                        )
                    nc.vector.tensor_reduce(
                        fin[:, 4 * G : 4 * G + 4], xg[:, :, 0:w], axis=AX.X, op=OP.min
                    )

                # ---- col direction finalization ---------------------------
                CH = min(2048, n)
                for h in range(n // CH):
                    psT = ps.tile([128, CH], F16, tag="mm")
                    nt = CH // 128
                    for t in range(nt):
                        nc.tensor.transpose(
                            psT[:, t * 128 : (t + 1) * 128],
                            colacc[:, h * CH + t * 128 : h * CH + (t + 1) * 128],
                            ident,
                        )
                    nc.vector.tensor_reduce(
                        fin[:, n_m + h * nt : n_m + (h + 1) * nt],
                        psT[:, :].rearrange("q (t p) -> q t p", p=128),
                        axis=AX.X,
                        op=OP.min,
                    )

                # ---- partition sums via ones-matmul -----------------------
                fsum = ps.tile([1, 2 * n_m], F32, tag="mm")
                nc.tensor.matmul(fsum, ones, fin, start=True, stop=True)
                res = sb.tile([1, 2], F32, tag="res")
                nc.vector.tensor_reduce(
                    res[0:1, 0:1], fsum[0:1, 0:n_m], axis=AX.X, op=OP.add
                )
                nc.vector.tensor_reduce(
                    res[0:1, 1:2], fsum[0:1, n_m : 2 * n_m], axis=AX.X, op=OP.add
                )
                nc.sync.dma_start(out_d.ap()[b : b + 1, :], res)

    nc.compile()
    return nc


def prep_inputs(pred: np.ndarray, target: np.ndarray):
    """Host-side layout prep: fp16 hi/lo split operands for the K=10 matmul."""
    B, n, _ = pred.shape
    pred = pred.astype(np.float32)
    target = target.astype(np.float32)

    def f16(x):
        return x.astype(np.float16)

    ax, ay = pred[..., 0], pred[..., 1]
    bx, by = target[..., 0], target[..., 1]
    a2 = ax * ax + ay * ay
    b2 = bx * bx + by * by
    one = np.ones((B, n), dtype=np.float16)

    h_ax, h_ay = f16(ax), f16(ay)
    l_ax = f16(ax - h_ax.astype(np.float32))
    l_ay = f16(ay - h_ay.astype(np.float32))
    h_bx, h_by = f16(bx), f16(by)
    l_bx = f16(bx - h_bx.astype(np.float32))
    l_by = f16(by - h_by.astype(np.float32))
    h_a2 = f16(a2)
    l_a2 = f16(a2 - h_a2.astype(np.float32))
    h_b2 = f16(b2)
    l_b2 = f16(b2 - h_b2.astype(np.float32))

    m2 = np.float16(-2.0)
    L = np.stack(
        [m2 * h_ax, m2 * h_ay, m2 * h_ax, m2 * h_ay, m2 * l_ax, m2 * l_ay,
         h_a2, l_a2, one, one],
        axis=1,
    )  # [B, KR, n] fp16
    R = np.stack(
        [h_bx, h_by, l_bx, l_by, h_bx, h_by, one, one, h_b2, l_b2], axis=1
    )  # [B, KR, n] fp16

    # predQ[b, g, r, G*128+c] = L[b, r, (4G+g)*128+c]
    n_gm = n // 512
    L5 = L.reshape(B, KR, n_gm, 4, 128)             # [b, r, G, g, c]
    predQ = np.ascontiguousarray(
        L5.transpose(0, 3, 1, 2, 4).reshape(B, 4, KR, n // 4)
    )
    targQ = np.ascontiguousarray(np.broadcast_to(R[:, None], (B, 4, KR, n)))
    return predQ, targQ


_CACHE: dict = {}


def _get_nc(nb: int, n: int) -> bacc.Bacc:
    key = (nb, n)
    if key not in _CACHE:
        _CACHE[key] = build_chamfer(nb, n)
    return _CACHE[key]


def run_device(pred: np.ndarray, target: np.ndarray, trace: bool = False):
    """Run on the 8 NeuronCores. Returns (out[2] float32, BassKernelResults)."""
    B, n, _ = pred.shape
    nb = B // N_CORES
    nc = _get_nc(nb, n)
    predQ, targQ = prep_inputs(pred, target)
    in_maps = [
        {
            "predQ": predQ[c * nb : (c + 1) * nb],
            "targQ": targQ[c * nb : (c + 1) * nb],
        }
        for c in range(N_CORES)
    ]
    res = run_bass_kernel_spmd(nc, in_maps, core_ids=list(range(N_CORES)), trace=trace)
    partial = np.stack([r["out"] for r in res.results])  # [cores, nb, 2]
    total = partial.reshape(-1, 2).sum(axis=0, dtype=np.float64)
    denom = float(n * B)
    out = (total / denom).astype(np.float32)
    return out, res


def kernel(pred: np.ndarray, target: np.ndarray) -> np.ndarray:
    pred = np.asarray(pred, dtype=np.float32)
    target = np.asarray(target, dtype=np.float32)
    out, _ = run_device(pred, target, trace=False)
    return out


# revision 27
# speedup vs baseline: 1.0004x; 1.0004x over previous
"""Chamfer distance kernel for Trainium2 (8 NeuronCores, data-parallel over batch).

Per core (NB=2 batches of the global B=16):
  The [N, N] squared-distance matrix is computed tilewise on the TensorEngine
  with a split-precision K=10 fp16 matmul (fp16 hi/lo decomposition of the
  fp32 inputs; fp16 products are exact and accumulate in fp32 PSUM, so the
  result matches fp32 to ~1e-6 absolute while running 4x faster than fp32
  matmuls):
      d2[i,j] = a2_i + b2_j - 2 a_i.b_j
      rows:  [-2h_ax, -2h_ay, -2h_ax, -2h_ay, -2l_ax, -2l_ay, h_a2, l_a2, 1, 1]
      cols:  [ h_bx,   h_by,   l_bx,   l_by,   h_bx,   h_by,   1,   1, h_b2, l_b2]
  Four 128-row tiles are packed into the four 32-row groups of the PE array
  (tile_position) so their matmuls run concurrently.
  ScalarE copies PSUM fp32 -> SBUF fp16 (the d2 values are >= 0 and small, so
  fp16 error is relative, ~2^-11).
  Row direction (pred->target): in-place fp16 min-tree along the free axis +
  reduce_min. Col direction (target->pred): VectorE pre-mins the 4 row-tiles
  pairwise and accumulates into colacc[128, N]; finally PE-transposed 128x128
  blocks + reduce_min. Partition sums via a ones-vector matmul.
Each core returns [NB, 2] partial sums; the host sums across cores and
divides by N*B. Host does O(N) layout prep only; all O(N^2) work is on device.
"""

import os

import numpy as np

# The axon NTFF-profiling hook module (antenv.axon_hooks) is absent in this
# image; if BASS_TRACE happens to be set in the environment, the trace path
# would crash on import. Never trace from the kernel itself.
os.environ["BASS_NEVER_TRACE"] = "1"

import concourse.bass as bass
import concourse.mybir as mybir
from concourse import bacc
from concourse.tile import TileContext
from concourse.masks import make_identity
from concourse.bass_utils import run_bass_kernel_spmd

F32 = mybir.dt.float32
F16 = mybir.dt.float16
AX = mybir.AxisListType
OP = mybir.AluOpType

N_CORES = 8
KR = 10                   # split-precision contraction depth
FBIG = 60000.0            # fp16-representable "infinity"


def build_chamfer(nb: int, n: int) -> bacc.Bacc:
    """Build the per-core Bass program: nb batches of n points (2-D each)."""
    assert n % 512 == 0
    n_m = n // 128            # 128-row tiles
    n_g = n_m // 4            # groups of 4 row-tiles
    JC = 512                  # matmul moving-operand width
    n_j = n // JC

    nc = bacc.Bacc(
        "TRN2", target_bir_lowering=False, debug=False, enable_asserts=False
    )
    # predQ: per group-slot g (0..3): the KR lhsT rows of row-tile m = 4*G + g,
    # at partitions 32g..32g+KR-1, columns G*128..(G+1)*128.
    predQ_d = nc.dram_tensor("predQ", [nb, 4, KR, n // 4], F16, kind="ExternalInput")
    # targQ: the KR rhs rows replicated at partitions 32g..32g+KR-1.
    targQ_d = nc.dram_tensor("targQ", [nb, 4, KR, n], F16, kind="ExternalInput")
    out_d = nc.dram_tensor("out", [nb, 2], F32, kind="ExternalOutput")

    with TileContext(nc) as tc:
        with (
            tc.tile_pool(name="persist", bufs=1) as pp,
            tc.tile_pool(name="sb", bufs=1) as sb,
            tc.tile_pool(name="sbin", bufs=2) as sbin,
            tc.tile_pool(name="sbx", bufs=3) as sbx,
            tc.tile_pool(name="sbc", bufs=2) as sbc,
            tc.tile_pool(name="ps", bufs=2, space="PSUM") as ps,
        ):
            ident = pp.tile([128, 128], F16)
            make_identity(nc, ident)
            ones = pp.tile([128, 1], F32)
            nc.vector.memset(ones, 1.0)

            for b in range(nb):
                predQ = sbin.tile([128, n // 4], F16, tag="predQ")
                targQ = sbin.tile([128, n], F16, tag="targQ")
                for g in range(4):
                    nc.sync.dma_start(
                        predQ[32 * g : 32 * g + KR, :], predQ_d.ap()[b, g]
                    )
                    nc.sync.dma_start(
                        targQ[32 * g : 32 * g + KR, :], targQ_d.ap()[b, g]
                    )

                colacc = sb.tile([128, n], F16, tag="colacc")
                nc.gpsimd.memset(colacc, FBIG)
                # fin columns: [0, n_m) = rowmins, [n_m, 2*n_m) = colmins
                fin = sb.tile([128, 2 * n_m], F32, tag="fin")

                for G in range(n_g):
                    # xg: fp16 d2 rows for the 4 row-tiles of this group
                    xg = sbx.tile([128, 4, n], F16, tag="xg")
                    for j in range(n_j):
                        pst = ps.tile([128, 4 * JC], F32, tag="mm")
                        for g in range(4):
                            nc.tensor.matmul(
                                pst[:, g * JC : (g + 1) * JC],
                                predQ[32 * g : 32 * g + KR, G * 128 : (G + 1) * 128],
                                targQ[32 * g : 32 * g + KR, j * JC : (j + 1) * JC],
                                start=True,
                                stop=True,
                                tile_position=(32 * g, 0),
                            )
                        # PSUM fp32 [128, 4*JC] -> SBUF fp16, strided over xg.
                        nc.scalar.copy(xg[:, :, j * JC : (j + 1) * JC], pst)

                    # col direction: pre-min the 4 row-tiles pairwise (before
                    # the in-place row tree destroys xg), then fold into
                    # colacc. c2 is computed in-place in c1's first half.
                    c1 = sbc.tile([128, 2, n], F16, tag="c1")
                    nc.vector.tensor_tensor(
                        c1, xg[:, 0:2, :], xg[:, 2:4, :], op=OP.min
                    )
                    nc.vector.tensor_tensor(
                        c1[:, 0, :], c1[:, 0, :], c1[:, 1, :], op=OP.min
                    )
                    nc.vector.tensor_tensor(colacc, colacc, c1[:, 0, :], op=OP.min)

                    # row direction: in-place min-tree over [128, 4, n]
                    w = n // 2
                    nc.vector.tensor_tensor(
                        xg[:, :, 0:w], xg[:, :, 0:w], xg[:, :, w : 2 * w], op=OP.min
                    )
                    while w > 256:
                        w //= 2
                        nc.vector.tensor_tensor(
                            xg[:, :, 0:w], xg[:, :, 0:w], xg[:, :, w : 2 * w],
                            op=OP.min,
                        )
                    nc.vector.tensor_reduce(
                        fin[:, 4 * G : 4 * G + 4], xg[:, :, 0:w], axis=AX.X, op=OP.min
                    )

                # ---- col direction finalization ---------------------------
                CH = min(2048, n)
                for h in range(n // CH):
                    psT = ps.tile([128, CH], F16, tag="mm")
                    nt = CH // 128
                    for t in range(nt):
                        nc.tensor.transpose(
                            psT[:, t * 128 : (t + 1) * 128],
                            colacc[:, h * CH + t * 128 : h * CH + (t + 1) * 128],
                            ident,
                        )
                    nc.vector.tensor_reduce(
                        fin[:, n_m + h * nt : n_m + (h + 1) * nt],
                        psT[:, :].rearrange("q (t p) -> q t p", p=128),
                        axis=AX.X,
                        op=OP.min,
                    )

                # ---- partition sums via ones-matmul -----------------------
                fsum = ps.tile([1, 2 * n_m], F32, tag="mm")
                nc.tensor.matmul(fsum, ones, fin, start=True, stop=True)
                res = sb.tile([1, 2], F32, tag="res")
                nc.vector.tensor_reduce(
                    res[0:1, 0:1], fsum[0:1, 0:n_m], axis=AX.X, op=OP.add
                )
                nc.vector.tensor_reduce(
                    res[0:1, 1:2], fsum[0:1, n_m : 2 * n_m], axis=AX.X, op=OP.add
                )
                nc.sync.dma_start(out_d.ap()[b : b + 1, :], res)

    nc.compile()
    return nc


def prep_inputs(pred: np.ndarray, target: np.ndarray):
    """Host-side layout prep: fp16 hi/lo split operands for the K=10 matmul."""
    B, n, _ = pred.shape
    pred = pred.astype(np.float32)
    target = target.astype(np.float32)

    def f16(x):
        return x.astype(np.float16)

    ax, ay = pred[..., 0], pred[..., 1]
    bx, by = target[..., 0], target[..., 1]
    a2 = ax * ax + ay * ay
    b2 = bx * bx + by * by
    one = np.ones((B, n), dtype=np.float16)

    h_ax, h_ay = f16(ax), f16(ay)
    l_ax = f16(ax - h_ax.astype(np.float32))
    l_ay = f16(ay - h_ay.astype(np.float32))
    h_bx, h_by = f16(bx), f16(by)
    l_bx = f16(bx - h_bx.astype(np.float32))
    l_by = f16(by - h_by.astype(np.float32))
    h_a2 = f16(a2)
    l_a2 = f16(a2 - h_a2.astype(np.float32))
    h_b2 = f16(b2)
    l_b2 = f16(b2 - h_b2.astype(np.float32))

    m2 = np.float16(-2.0)
    L = np.stack(
        [m2 * h_ax, m2 * h_ay, m2 * h_ax, m2 * h_ay, m2 * l_ax, m2 * l_ay,
         h_a2, l_a2, one, one],
        axis=1,
    )  # [B, KR, n] fp16
    R = np.stack(
        [h_bx, h_by, l_bx, l_by, h_bx, h_by, one, one, h_b2, l_b2], axis=1
    )  # [B, KR, n] fp16

    # predQ[b, g, r, G*128+c] = L[b, r, (4G+g)*128+c]
    n_gm = n // 512
    L5 = L.reshape(B, KR, n_gm, 4, 128)             # [b, r, G, g, c]
    predQ = np.ascontiguousarray(
        L5.transpose(0, 3, 1, 2, 4).reshape(B, 4, KR, n // 4)
    )
    targQ = np.ascontiguousarray(np.broadcast_to(R[:, None], (B, 4, KR, n)))
    return predQ, targQ


_CACHE: dict = {}


def _get_nc(nb: int, n: int) -> bacc.Bacc:
    key = (nb, n)
    if key not in _CACHE:
        _CACHE[key] = build_chamfer(nb, n)
    return _CACHE[key]


def run_device(pred: np.ndarray, target: np.ndarray, trace: bool = False):
    """Run on the 8 NeuronCores. Returns (out[2] float32, BassKernelResults)."""
    B, n, _ = pred.shape
    nb = B // N_CORES
    nc = _get_nc(nb, n)
    predQ, targQ = prep_inputs(pred, target)
    in_maps = [
        {
            "predQ": predQ[c * nb : (c + 1) * nb],
            "targQ": targQ[c * nb : (c + 1) * nb],
        }
        for c in range(N_CORES)
    ]
    res = run_bass_kernel_spmd(nc, in_maps, core_ids=list(range(N_CORES)), trace=trace)
    partial = np.stack([r["out"] for r in res.results])  # [cores, nb, 2]
    total = partial.reshape(-1, 2).sum(axis=0, dtype=np.float64)
    denom = float(n * B)
    out = (total / denom).astype(np.float32)
    return out, res


def kernel(pred: np.ndarray, target: np.ndarray) -> np.ndarray:
    pred = np.asarray(pred, dtype=np.float32)
    target = np.asarray(target, dtype=np.float32)
    out, _ = run_device(pred, target, trace=False)
    return out


# revision 29
# speedup vs baseline: 1.0199x; 1.0195x over previous
"""Chamfer distance kernel for Trainium2 (8 NeuronCores, data-parallel over batch).

Per core (NB=2 batches of the global B=16):
  The [N, N] squared-distance matrix is computed tilewise on the TensorEngine
  with a split-precision K=10 fp16 matmul (fp16 hi/lo decomposition of the
  fp32 inputs; fp16 products are exact and accumulate in fp32 PSUM, so the
  result matches fp32 to ~1e-6 absolute while running 4x faster than fp32
  matmuls):
      d2[i,j] = a2_i + b2_j - 2 a_i.b_j
      rows:  [-2h_ax, -2h_ay, -2h_ax, -2h_ay, -2l_ax, -2l_ay, h_a2, l_a2, 1, 1]
      cols:  [ h_bx,   h_by,   l_bx,   l_by,   h_bx,   h_by,   1,   1, h_b2, l_b2]
  Four 128-row tiles are packed into the four 32-row groups of the PE array
  (tile_position) so their matmuls run concurrently.
  ScalarE copies PSUM fp32 -> SBUF fp16 (the d2 values are >= 0 and small, so
  fp16 error is relative, ~2^-11).
  Row direction (pred->target): in-place fp16 min-tree along the free axis +
  reduce_min. Col direction (target->pred): VectorE pre-mins the 4 row-tiles
  pairwise and accumulates into colacc[128, N]; finally PE-transposed 128x128
  blocks + reduce_min. Partition sums via a ones-vector matmul.
Each core returns [NB, 2] partial sums; the host sums across cores and
divides by N*B. Host does O(N) layout prep only; all O(N^2) work is on device.
"""

import os

import numpy as np

# The axon NTFF-profiling hook module (antenv.axon_hooks) is absent in this
# image; if BASS_TRACE happens to be set in the environment, the trace path
# would crash on import. Never trace from the kernel itself.
os.environ["BASS_NEVER_TRACE"] = "1"

import concourse.bass as bass
import concourse.mybir as mybir
from concourse import bacc
from concourse.tile import TileContext
from concourse.masks import make_identity
from concourse.bass_utils import run_bass_kernel_spmd

F32 = mybir.dt.float32
F16 = mybir.dt.float16
AX = mybir.AxisListType
OP = mybir.AluOpType

N_CORES = 8
KR = 10                   # split-precision contraction depth
FBIG = 60000.0            # fp16-representable "infinity"


def build_chamfer(nb: int, n: int) -> bacc.Bacc:
    """Build the per-core Bass program: nb batches of n points (2-D each)."""
    assert n % 512 == 0
    n_m = n // 128            # 128-row tiles
    n_g = n_m // 4            # groups of 4 row-tiles
    JC = 512                  # matmul moving-operand width
    n_j = n // JC

    nc = bacc.Bacc(
        "TRN2", target_bir_lowering=False, debug=False, enable_asserts=False
    )
    # predQ: per group-slot g (0..3): the KR lhsT rows of row-tile m = 4*G + g,
    # at partitions 32g..32g+KR-1, columns G*128..(G+1)*128.
    predQ_d = nc.dram_tensor("predQ", [nb, 4, KR, n // 4], F16, kind="ExternalInput")
    # targQ: the KR rhs rows replicated at partitions 32g..32g+KR-1.
    targQ_d = nc.dram_tensor("targQ", [nb, 4, KR, n], F16, kind="ExternalInput")
    out_d = nc.dram_tensor("out", [nb, 2], F32, kind="ExternalOutput")

    with TileContext(nc) as tc:
        with (
            tc.tile_pool(name="persist", bufs=1) as pp,
            tc.tile_pool(name="sb", bufs=1) as sb,
            tc.tile_pool(name="sbin", bufs=2) as sbin,
            tc.tile_pool(name="sbx", bufs=3) as sbx,
            tc.tile_pool(name="sbc", bufs=2) as sbc,
            tc.tile_pool(name="ps", bufs=2, space="PSUM") as ps,
        ):
            ident = pp.tile([128, 128], F16)
            make_identity(nc, ident)
            ones = pp.tile([128, 1], F32)
            nc.vector.memset(ones, 1.0)

            for b in range(nb):
                predQ = sbin.tile([128, n // 4], F16, tag="predQ")
                targQ = sbin.tile([128, n], F16, tag="targQ")
                for g in range(4):
                    nc.sync.dma_start(
                        predQ[32 * g : 32 * g + KR, :], predQ_d.ap()[b, g]
                    )
                    nc.sync.dma_start(
                        targQ[32 * g : 32 * g + KR, :], targQ_d.ap()[b, g]
                    )

                colacc = sb.tile([128, n], F16, tag="colacc")
                nc.gpsimd.memset(colacc, FBIG)
                # fin columns: [0, n_m) = rowmins, [n_m, 2*n_m) = colmins
                fin = sb.tile([128, 2 * n_m], F32, tag="fin")

                for G in range(n_g):
                    # xg: fp16 d2 rows for the 4 row-tiles of this group
                    xg = sbx.tile([128, 4, n], F16, tag="xg")
                    # For the first group of each batch, compute the row-min
                    # incrementally per chunk (instead of the post-hoc tree)
                    # so VectorE has work while the first xg fills -- this
                    # removes the ~25us startup / ~15us batch-boundary stalls.
                    inc = G == 0 and n_j > 1
                    if inc:
                        racc = sbx.tile([128, 4, JC], F16, tag="racc")
                    for j in range(n_j):
                        pst = ps.tile([128, 4 * JC], F32, tag="mm")
                        for g in range(4):
                            nc.tensor.matmul(
                                pst[:, g * JC : (g + 1) * JC],
                                predQ[32 * g : 32 * g + KR, G * 128 : (G + 1) * 128],
                                targQ[32 * g : 32 * g + KR, j * JC : (j + 1) * JC],
                                start=True,
                                stop=True,
                                tile_position=(32 * g, 0),
                            )
                        # PSUM fp32 [128, 4*JC] -> SBUF fp16, strided over xg.
                        nc.scalar.copy(xg[:, :, j * JC : (j + 1) * JC], pst)
                        if inc:
                            sl = xg[:, :, j * JC : (j + 1) * JC]
                            if j == 0:
                                nc.vector.tensor_copy(racc, sl)
                            else:
                                nc.vector.tensor_tensor(racc, racc, sl, op=OP.min)

                    # col direction: pre-min the 4 row-tiles pairwise (before
                    # the in-place row tree destroys xg), then fold into
                    # colacc. c2 is computed in-place in c1's first half.
                    c1 = sbc.tile([128, 2, n], F16, tag="c1")
                    nc.vector.tensor_tensor(
                        c1, xg[:, 0:2, :], xg[:, 2:4, :], op=OP.min
                    )
                    nc.vector.tensor_tensor(
                        c1[:, 0, :], c1[:, 0, :], c1[:, 1, :], op=OP.min
                    )
                    nc.vector.tensor_tensor(colacc, colacc, c1[:, 0, :], op=OP.min)

                    # row direction
                    if inc:
                        nc.vector.tensor_reduce(
                            fin[:, 4 * G : 4 * G + 4], racc, axis=AX.X, op=OP.min
                        )
                    else:
                        # in-place min-tree over [128, 4, n]
                        w = n // 2
                        nc.vector.tensor_tensor(
                            xg[:, :, 0:w], xg[:, :, 0:w], xg[:, :, w : 2 * w],
                            op=OP.min,
                        )
                        while w > 256:
                            w //= 2
                            nc.vector.tensor_tensor(
                                xg[:, :, 0:w], xg[:, :, 0:w], xg[:, :, w : 2 * w],
                                op=OP.min,
                            )
                        nc.vector.tensor_reduce(
                            fin[:, 4 * G : 4 * G + 4], xg[:, :, 0:w],
                            axis=AX.X, op=OP.min,
                        )

                # ---- col direction finalization ---------------------------
                CH = min(2048, n)
                for h in range(n // CH):
                    psT = ps.tile([128, CH], F16, tag="mm")
                    nt = CH // 128
                    for t in range(nt):
                        nc.tensor.transpose(
                            psT[:, t * 128 : (t + 1) * 128],
                            colacc[:, h * CH + t * 128 : h * CH + (t + 1) * 128],
                            ident,
                        )
                    nc.vector.tensor_reduce(
                        fin[:, n_m + h * nt : n_m + (h + 1) * nt],
                        psT[:, :].rearrange("q (t p) -> q t p", p=128),
                        axis=AX.X,
                        op=OP.min,
                    )

                # ---- partition sums via ones-matmul -----------------------
                fsum = ps.tile([1, 2 * n_m], F32, tag="mm")
                nc.tensor.matmul(fsum, ones, fin, start=True, stop=True)
                res = sb.tile([1, 2], F32, tag="res")
                nc.vector.tensor_reduce(
                    res[0:1, 0:1], fsum[0:1, 0:n_m], axis=AX.X, op=OP.add
                )
                nc.vector.tensor_reduce(
                    res[0:1, 1:2], fsum[0:1, n_m : 2 * n_m], axis=AX.X, op=OP.add
                )
                nc.sync.dma_start(out_d.ap()[b : b + 1, :], res)

    nc.compile()
    return nc


def prep_inputs(pred: np.ndarray, target: np.ndarray):
    """Host-side layout prep: fp16 hi/lo split operands for the K=10 matmul."""
    B, n, _ = pred.shape
    pred = pred.astype(np.float32)
    target = target.astype(np.float32)

    def f16(x):
        return x.astype(np.float16)

    ax, ay = pred[..., 0], pred[..., 1]
    bx, by = target[..., 0], target[..., 1]
    a2 = ax * ax + ay * ay
    b2 = bx * bx + by * by
    one = np.ones((B, n), dtype=np.float16)

    h_ax, h_ay = f16(ax), f16(ay)
    l_ax = f16(ax - h_ax.astype(np.float32))
    l_ay = f16(ay - h_ay.astype(np.float32))
    h_bx, h_by = f16(bx), f16(by)
    l_bx = f16(bx - h_bx.astype(np.float32))
    l_by = f16(by - h_by.astype(np.float32))
    h_a2 = f16(a2)
    l_a2 = f16(a2 - h_a2.astype(np.float32))
    h_b2 = f16(b2)
    l_b2 = f16(b2 - h_b2.astype(np.float32))

    m2 = np.float16(-2.0)
    L = np.stack(
        [m2 * h_ax, m2 * h_ay, m2 * h_ax, m2 * h_ay, m2 * l_ax, m2 * l_ay,
         h_a2, l_a2, one, one],
        axis=1,
    )  # [B, KR, n] fp16
    R = np.stack(
        [h_bx, h_by, l_bx, l_by, h_bx, h_by, one, one, h_b2, l_b2], axis=1
    )  # [B, KR, n] fp16

    # predQ[b, g, r, G*128+c] = L[b, r, (4G+g)*128+c]
    n_gm = n // 512
    L5 = L.reshape(B, KR, n_gm, 4, 128)             # [b, r, G, g, c]
    predQ = np.ascontiguousarray(
        L5.transpose(0, 3, 1, 2, 4).reshape(B, 4, KR, n // 4)
    )
    targQ = np.ascontiguousarray(np.broadcast_to(R[:, None], (B, 4, KR, n)))
    return predQ, targQ


_CACHE: dict = {}


def _get_nc(nb: int, n: int) -> bacc.Bacc:
    key = (nb, n)
    if key not in _CACHE:
        _CACHE[key] = build_chamfer(nb, n)
    return _CACHE[key]


def run_device(pred: np.ndarray, target: np.ndarray, trace: bool = False):
    """Run on the 8 NeuronCores. Returns (out[2] float32, BassKernelResults)."""
    B, n, _ = pred.shape
    nb = B // N_CORES
    nc = _get_nc(nb, n)
    predQ, targQ = prep_inputs(pred, target)
    in_maps = [
        {
            "predQ": predQ[c * nb : (c + 1) * nb],
            "targQ": targQ[c * nb : (c + 1) * nb],
        }
        for c in range(N_CORES)
    ]
    res = run_bass_kernel_spmd(nc, in_maps, core_ids=list(range(N_CORES)), trace=trace)
    partial = np.stack([r["out"] for r in res.results])  # [cores, nb, 2]
    total = partial.reshape(-1, 2).sum(axis=0, dtype=np.float64)
    denom = float(n * B)
    out = (total / denom).astype(np.float32)
    return out, res


def kernel(pred: np.ndarray, target: np.ndarray) -> np.ndarray:
    pred = np.asarray(pred, dtype=np.float32)
    target = np.asarray(target, dtype=np.float32)
    out, _ = run_device(pred, target, trace=False)
    return out
